# revision 7
# baseline (speedup 1.0000x reference)
"""GQA kernel for Trainium2, sharded over 8 NeuronCores.

Sharding: data-parallel over batch (2) x tensor-parallel over kv_heads (4).
Core c = b*4 + h computes the full attention output partial
    Y_bh = softmax(causal((Q_b @ Wq_eff_h) @ (K_b @ Wk_h)^T / sqrt(dk))) @ (V_b @ Wv_h) @ Wo_h
and the host sums the 4 head partials per batch (the "all-reduce after Wo").
The GQA group-sum-before-softmax quirk folds into the weights:
    Wq_eff_h = sum_g Wq[:, (g*KV+h)*dk : ...].

Precision/bandwidth plan (validated by simulation against the reference):
  - Q^T, K^T stream as TWO e4m3 planes (main + exact-scale residual), same
    bytes as fp16, but enabling fp8 DoubleRow matmuls: each projection is
    3 DoubleRow terms (x8*w8 + x8*wr + xr*w8) contracting 256 rows per
    instruction at 0.5 cyc/row -> 25% fewer PE cycles than fp16 with
    ~fp16 accuracy (sim: 0.19% max rel err).
  - V^T streams as a single e3m4 plane (half the bytes of fp16); wv fp16
    stationary (mixed-dtype matmul). Sim: 1.58% max rel err total, under
    the 2e-2 harness gate. Set VT_E3=False to fall back to fp16 V.
  - q/k evictions carry x32/x64 weight scales, folded into the exp scale.
  - Scores, softmax, PV, and Y = O @ Wo all stay fp16 (fp32 PSUM).

Schedule: K^T arrives row-major in dc-pairs (kproj chases the wire);
V^T and Q^T arrive column-block-major (512 seq positions per block) so
vT/qT blocks complete early; per q-chunk j the chain
qproj(j) -> scores(j,*) -> PV(j) -> Y(j-1) -> y DMA runs as soon as
block j lands, overlapping the y output DMA with the tail of the input
stream. DMA wire is the binding resource (~30MB at ~350GB/s).
"""
import sys
sys.path.insert(0, '/opt/trn_rl_repo')
import math
import numpy as np
import ml_dtypes

import concourse.bass as bass
import concourse.mybir as mybir
import concourse.tile as tile
from concourse import bacc
from concourse import bass_utils
from concourse.masks import make_identity

FP32 = mybir.dt.float32
FP16 = mybir.dt.float16
E4 = mybir.dt.float8e4
E3 = mybir.dt.float8e3
NE4 = ml_dtypes.float8_e4m3
NE3 = ml_dtypes.float8_e3m4
DR = mybir.MatmulPerfMode.DoubleRow

B, L, D = 2, 2048, 2048
Q_HEADS, KV_HEADS, DK, DV = 16, 4, 128, 128
GROUPS = Q_HEADS // KV_HEADS
P = 128
CH = 512                 # lq/lk block width
NJ = L // CH             # 4 query chunks
NDC = D // P             # 16 contraction tiles
NPAIR = NDC // 2         # 8 DoubleRow contraction pairs
NB = L // CH             # 4 lk blocks for vproj
SQ, SK = 32.0, 64.0      # weight scales (powers of 2; folded into exp scale)
SCALE_EXP = (1.0 / math.sqrt(DK)) / (SQ * SK)
EBIAS = -8.0 * math.log(2.0)   # exp output scaled by 2^-8; cancels in softmax
VT_E3 = True             # V^T as e3m4 single plane (False -> fp16)
VT_DT, VT_NP = (E3, NE3) if VT_E3 else (FP16, np.float16)


def _build():
    nc = bacc.Bacc(trn_type="TRN2")
    kt8_d = nc.dram_tensor("kt8", (D, L), E4, kind="ExternalInput")
    ktr_d = nc.dram_tensor("ktr", (D, L), E4, kind="ExternalInput")
    qt8_d = nc.dram_tensor("qt8", (NJ, D, CH), E4, kind="ExternalInput")
    qtr_d = nc.dram_tensor("qtr", (NJ, D, CH), E4, kind="ExternalInput")
    vt_d = nc.dram_tensor("vt", (NB, D, CH), VT_DT, kind="ExternalInput")
    wq8_d = nc.dram_tensor("wq8", (P, NDC, DK), E4, kind="ExternalInput")
    wqr_d = nc.dram_tensor("wqr", (P, NDC, DK), E4, kind="ExternalInput")
    wk8_d = nc.dram_tensor("wk8", (P, NDC, DK), E4, kind="ExternalInput")
    wkr_d = nc.dram_tensor("wkr", (P, NDC, DK), E4, kind="ExternalInput")
    wv_d = nc.dram_tensor("wv", (P, NDC, DV), FP16, kind="ExternalInput")
    wo_d = nc.dram_tensor("wo", (DV, D), FP16, kind="ExternalInput")
    mask_d = nc.dram_tensor("mask", (P, NJ * CH), FP16, kind="ExternalInput")
    y_d = nc.dram_tensor("y", (L, D), FP16, kind="ExternalOutput")

    with tile.TileContext(nc) as tc:
        with (
            tc.tile_pool(name="const", bufs=1) as const,
            tc.tile_pool(name="wpool", bufs=1) as wpool,
            tc.tile_pool(name="kxs", bufs=2) as kxs,
            tc.tile_pool(name="qxs", bufs=2) as qxs,
            tc.tile_pool(name="vxs", bufs=2) as vxs,
            tc.tile_pool(name="proj", bufs=1) as proj,
            tc.tile_pool(name="etp", bufs=2) as etp,
            tc.tile_pool(name="ev", bufs=3) as ev_pool,
            tc.tile_pool(name="ps", bufs=7, space="PSUM") as ps,
        ):
            ident = const.tile([P, P], FP16)
            make_identity(nc, ident[:])
            ones = const.tile([P, P], FP16)
            nc.vector.memset(ones[:], 1.0)
            ones2 = const.tile([P, 256], FP16)
            nc.vector.memset(ones2[:], 1.0)
            ebias = const.tile([P, 1], FP32)
            nc.vector.memset(ebias[:], EBIAS)
            maskt = const.tile([P, NJ * CH], FP16)

            kT = proj.tile([P, L], FP16, tag="kT")
            qT = proj.tile([P, L], FP16, tag="qT")
            vT = proj.tile([P, L], FP16, tag="vT")
            v_nat = proj.tile([P, L], FP16, tag="v_nat")
            oT = proj.tile([P, L], FP16, tag="oT")
            rinv_all = proj.tile([P, NJ * CH], FP32, tag="rinv_all")

            wk8 = wpool.tile([P, NDC, DK], E4, tag="wk8")
            wkr = wpool.tile([P, NDC, DK], E4, tag="wkr")
            wq8 = wpool.tile([P, NDC, DK], E4, tag="wq8")
            wqr = wpool.tile([P, NDC, DK], E4, tag="wqr")
            wv = wpool.tile([P, NDC, DV], FP16, tag="wv")
            wo_sb = wpool.tile([DV, D], FP16, tag="wo")

            warm = ps.tile([P, 256], FP32, tag="warm", bufs=1, name="warm")

            def fill():
                # keep the PE HAM window busy / p-state high during DMA waits
                nc.tensor.matmul(warm[:], ones[:], ones2[:], start=True, stop=True)

            # ---------- phase A: K projection (row-major dc-pair stream) ----
            nc.scalar.dma_start(wk8[:], wk8_d[:])
            nc.scalar.dma_start(wkr[:], wkr_d[:])
            nc.scalar.dma_start(maskt[:], mask_d[:])
            kaccs = [ps.tile([P, CH], FP32, tag="ps", name=f"kacc{b}")
                     for b in range(4)]
            for i in range(NPAIR):
                x8 = kxs.tile([P, 2, L], E4, tag="kx8", name="kx8")
                xr = kxs.tile([P, 2, L], E4, tag="kxr", name="kxr")
                nc.sync.dma_start(x8[:, 0, :], kt8_d[2 * i * P:(2 * i + 1) * P, :])
                nc.sync.dma_start(x8[:, 1, :], kt8_d[(2 * i + 1) * P:(2 * i + 2) * P, :])
                nc.sync.dma_start(xr[:, 0, :], ktr_d[2 * i * P:(2 * i + 1) * P, :])
                nc.sync.dma_start(xr[:, 1, :], ktr_d[(2 * i + 1) * P:(2 * i + 2) * P, :])
                wp8 = wk8[:, 2 * i:2 * i + 2, :]
                wpr = wkr[:, 2 * i:2 * i + 2, :]
                for blk in range(4):
                    xs8 = x8[:, :, blk * CH:(blk + 1) * CH]
                    xsr = xr[:, :, blk * CH:(blk + 1) * CH]
                    st0 = (i == 0)
                    sp1 = (i == NPAIR - 1)
                    nc.tensor.matmul(kaccs[blk][:], wp8, xs8,
                                     start=st0, stop=False, perf_mode=DR)
                    nc.tensor.matmul(kaccs[blk][:], wpr, xs8,
                                     start=False, stop=False, perf_mode=DR)
                    nc.tensor.matmul(kaccs[blk][:], wp8, xsr,
                                     start=False, stop=sp1, perf_mode=DR)
                fill()
            for blk in range(4):
                nc.vector.tensor_copy(kT[:, blk * CH:(blk + 1) * CH], kaccs[blk][:])

            # ---------- phase B: V projection (column-block stream) ---------
            nc.scalar.dma_start(wv[:], wv_d[:])
            for b in range(NB):
                vx = vxs.tile([P, NDC, CH], VT_DT, tag="vx", name="vx")
                for dc in range(NDC):
                    nc.sync.dma_start(vx[:, dc, :], vt_d[b, dc * P:(dc + 1) * P, :])
                vacc = ps.tile([P, CH], FP32, tag="ps", name=f"vacc{b}")
                for dc in range(NDC):
                    nc.tensor.matmul(vacc[:], wv[:, dc, :], vx[:, dc, :],
                                     start=(dc == 0), stop=(dc == NDC - 1))
                nc.vector.tensor_copy(vT[:, b * CH:(b + 1) * CH], vacc[:])
                for c in range(4 * b, 4 * b + 4):
                    tp = ps.tile([P, P], FP16, tag="ps", name="tp")
                    nc.tensor.transpose(tp[:], vT[:, c * P:(c + 1) * P], ident[:])
                    nc.scalar.copy(v_nat[:, c * P:(c + 1) * P], tp[:])
                fill()

            # ---------- phase C: per-j chains -------------------------------
            nc.scalar.dma_start(wq8[:], wq8_d[:])
            nc.scalar.dma_start(wqr[:], wqr_d[:])
            nc.scalar.dma_start(wo_sb[:], wo_d[:])

            ets = {}

            def qproj(j):
                q8 = qxs.tile([P, NDC, CH], E4, tag="qx8", name="qx8")
                qr = qxs.tile([P, NDC, CH], E4, tag="qxr", name="qxr")
                for dc in range(NDC):
                    nc.sync.dma_start(q8[:, dc, :], qt8_d[j, dc * P:(dc + 1) * P, :])
                for dc in range(NDC):
                    nc.sync.dma_start(qr[:, dc, :], qtr_d[j, dc * P:(dc + 1) * P, :])
                qacc = ps.tile([P, CH], FP32, tag="ps", name=f"qacc{j}")
                for i in range(NPAIR):
                    wp8 = wq8[:, 2 * i:2 * i + 2, :]
                    wpr = wqr[:, 2 * i:2 * i + 2, :]
                    xs8 = q8[:, 2 * i:2 * i + 2, :]
                    xsr = qr[:, 2 * i:2 * i + 2, :]
                    nc.tensor.matmul(qacc[:], wp8, xs8,
                                     start=(i == 0), stop=False, perf_mode=DR)
                    nc.tensor.matmul(qacc[:], wpr, xs8,
                                     start=False, stop=False, perf_mode=DR)
                    nc.tensor.matmul(qacc[:], wp8, xsr,
                                     start=False, stop=(i == NPAIR - 1), perf_mode=DR)
                nc.vector.tensor_copy(qT[:, j * CH:(j + 1) * CH], qacc[:])

            def scores(j):
                et_all = etp.tile([P, NDC, CH], FP16, tag="et", name="et")
                ets[j] = et_all
                rrep = ps.tile([P, CH], FP32, tag="ps", name=f"rrep{j}")
                for c in range(4 * j + 4):
                    st = ps.tile([P, CH], FP32, tag="ps", name="st")
                    nc.tensor.matmul(st[:], kT[:, c * P:(c + 1) * P],
                                     qT[:, j * CH:(j + 1) * CH],
                                     start=True, stop=True)
                    et = et_all[:, c, :]
                    nc.scalar.activation(et, st[:],
                                         mybir.ActivationFunctionType.Exp,
                                         bias=ebias[:], scale=SCALE_EXP)
                    d = c - 4 * j
                    if d >= 0:   # diagonal tile: zero out k > q
                        nc.vector.tensor_mul(et, et, maskt[:, d * CH:(d + 1) * CH])
                    nc.tensor.matmul(rrep[:], ones[:], et,
                                     start=(c == 0), stop=(c == 4 * j + 3))
                rinv = rinv_all[:, j * CH:(j + 1) * CH]
                nc.vector.reciprocal_approx_fast(rinv, rrep[:])

            def pv(j):
                et_all = ets[j]
                ot = ps.tile([P, CH], FP32, tag="ps", name="ot")
                for c in range(4 * j + 4):
                    nc.tensor.matmul(ot[:], v_nat[:, c * P:(c + 1) * P],
                                     et_all[:, c, :],
                                     start=(c == 0), stop=(c == 4 * j + 3))
                nc.vector.tensor_mul(oT[:, j * CH:(j + 1) * CH], ot[:],
                                     rinv_all[:, j * CH:(j + 1) * CH])

            def y_chunk(j):
                for t in range(CH // P):
                    lq0 = j * CH + t * P
                    yev = ev_pool.tile([P, D], FP16, tag="yev", name="yev")
                    for dch in range(D // CH):
                        yps = ps.tile([P, CH], FP32, tag="ps", name="yps")
                        nc.tensor.matmul(yps[:], oT[:, lq0:lq0 + P],
                                         wo_sb[:, dch * CH:(dch + 1) * CH],
                                         start=True, stop=True)
                        dst = yev[:, dch * CH:(dch + 1) * CH]
                        if dch % 2 == 0:
                            nc.vector.tensor_copy(dst, yps[:])
                        else:
                            nc.scalar.copy(dst, yps[:])
                    # gpsimd queue: keeps the sync queue free for qt prefetch
                    nc.gpsimd.dma_start(y_d[lq0:lq0 + P, :], yev[:])

            qproj(0)
            scores(0)
            for j in range(1, NJ):
                qproj(j)
                pv(j - 1)
                scores(j)
                y_chunk(j - 1)
            pv(NJ - 1)
            y_chunk(NJ - 1)
    nc.compile()
    return nc


_NC = None


def _get_nc():
    global _NC
    if _NC is None:
        _NC = _build()
    return _NC


def _pack_w(w):
    """(D, dk) fp32 -> [P, NDC, dk]: out[p, dc, m] = w[dc*128+p, m]"""
    return np.ascontiguousarray(w.reshape(NDC, P, -1).transpose(1, 0, 2))


def _two_plane(x):
    """fp32 array -> (main e4m3, residual e4m3 at the same scale)."""
    m = x.astype(NE4)
    r = (x - m.astype(np.float32)).astype(NE4)
    return m, r


def _col_blocks(xt, dt):
    """[D, L] -> contiguous (NJ, D, CH) in dtype dt."""
    return np.ascontiguousarray(
        xt.reshape(D, NJ, CH).transpose(1, 0, 2)).astype(dt)


def _make_in_maps(Q, K, V, Wq, Wk, Wv, Wo):
    f16 = np.float16
    Wq_eff = np.asarray(Wq, np.float32).reshape(D, GROUPS, KV_HEADS, DK).sum(axis=1)
    mask = np.zeros((P, NJ * CH), f16)
    for d in range(4):
        p = np.arange(P)[:, None]
        x = np.arange(CH)[None, :]
        mask[:, d * CH:(d + 1) * CH] = (128 * d + p <= x).astype(f16)
    acts = {}
    for b in range(B):
        qt = np.ascontiguousarray(np.asarray(Q[b], np.float32).T)
        kt = np.ascontiguousarray(np.asarray(K[b], np.float32).T)
        vt = np.ascontiguousarray(np.asarray(V[b], np.float32).T)
        kt8, ktr = _two_plane(kt)
        qt8f, qtrf = _two_plane(qt)
        acts[b] = {
            "kt8": kt8, "ktr": ktr,
            "qt8": _col_blocks(qt8f.astype(np.float32), NE4),
            "qtr": _col_blocks(qtrf.astype(np.float32), NE4),
            "vt": _col_blocks(vt, VT_NP),
        }
    Wk32, Wv32 = np.asarray(Wk, np.float32), np.asarray(Wv, np.float32)
    Wo32 = np.asarray(Wo, np.float32)
    in_maps = []
    for c in range(8):
        b, h = divmod(c, KV_HEADS)
        wq8, wqr = _two_plane(_pack_w(Wq_eff[:, h, :] * SQ))
        wk8, wkr = _two_plane(_pack_w(Wk32[:, h * DK:(h + 1) * DK] * SK))
        in_maps.append({
            **acts[b],
            "wq8": wq8, "wqr": wqr,
            "wk8": wk8, "wkr": wkr,
            "wv": _pack_w(Wv32[:, h * DV:(h + 1) * DV]).astype(f16),
            "wo": Wo32[h * DV:(h + 1) * DV, :].astype(f16),
            "mask": mask,
        })
    return in_maps


def _gather(results):
    Y = np.zeros((B, L, D), np.float32)
    for c in range(8):
        Y[c // KV_HEADS] += results[c]["y"].astype(np.float32)
    return Y


def kernel(Q, K, V, Wq, Wk, Wv, Wo):
    nc = _get_nc()
    in_maps = _make_in_maps(Q, K, V, Wq, Wk, Wv, Wo)
    res = bass_utils.run_bass_kernel_spmd(nc, in_maps, core_ids=list(range(8)))
    return _gather(res.results)


def _install_ntff_hook():
    """The agent image's antenv lacks axon_hooks; synthesize it so
    trace=True can reach the NTFF profiler in libaxon_pjrt.so."""
    import types
    import antenv
    if hasattr(antenv, "axon_hooks"):
        return
    mod = types.ModuleType("antenv.axon_hooks")
    _h = [None]
    mod.set_axon_ntff_profile_hook = lambda h: _h.__setitem__(0, h)
    mod.get_axon_ntff_profile_hook = lambda: _h[0]
    sys.modules["antenv.axon_hooks"] = mod
    antenv.axon_hooks = mod
    from trn_agent_boot.trn_boot import _ntff_profile_via_ctypes
    mod.set_axon_ntff_profile_hook(_ntff_profile_via_ctypes("/opt/axon/libaxon_pjrt.so"))


def kernel_traced(Q, K, V, Wq, Wk, Wv, Wo):
    """Like kernel() but profiles; returns (output, BassKernelResults)."""
    _install_ntff_hook()
    nc = _get_nc()
    in_maps = _make_in_maps(Q, K, V, Wq, Wk, Wv, Wo)
    res = bass_utils.run_bass_kernel_spmd(nc, in_maps, core_ids=list(range(8)),
                                          trace=True)
    return _gather(res.results), res


# revision 8
# speedup vs baseline: 1.1982x; 1.1982x over previous
"""GQA kernel for Trainium2, sharded over 8 NeuronCores.

Sharding: data-parallel over batch (2) x tensor-parallel over kv_heads (4).
Core c = b*4 + h computes the full attention output partial
    Y_bh = softmax(causal((Q_b @ Wq_eff_h) @ (K_b @ Wk_h)^T / sqrt(dk))) @ (V_b @ Wv_h) @ Wo_h
and the host sums the 4 head partials per batch (the "all-reduce after Wo").
The GQA group-sum-before-softmax quirk folds into the weights:
    Wq_eff_h = sum_g Wq[:, (g*KV+h)*dk : ...].

Bandwidth plan (validated in simulation + on hw):
  - V^T streams as a single e3m4 plane (half the bytes of fp16); wv fp16
    stationary (mixed-dtype matmul, verified on hw). Total max rel err
    1.58e-2, under the 2e-2 gate. VT_E3=False falls back to fp16 V.
  - Everything else fp16 with fp32 PSUM (DoubleRow fp8 measured to give
    no speedup on this hw, so 2-plane fp8 tricks are pointless).

Schedule: K^T arrives row-major (kproj chases the wire); V^T and Q^T
arrive column-block-major (512 positions per block) so vT blocks and
qT chunks complete early; per q-chunk j the chain
qproj(j) -> scores(j,*) -> PV(j-1) -> Y(j-1) -> y DMA runs as soon as
block j lands, overlapping the y output DMA with the input stream tail.
DMA wire is the binding resource (~32MB at ~350GB/s).
"""
import sys
sys.path.insert(0, '/opt/trn_rl_repo')
import math
import numpy as np
import ml_dtypes

import concourse.bass as bass
import concourse.mybir as mybir
import concourse.tile as tile
from concourse import bacc
from concourse import bass_utils
from concourse.masks import make_identity

FP32 = mybir.dt.float32
FP16 = mybir.dt.float16
E3 = mybir.dt.float8e3
NE3 = ml_dtypes.float8_e3m4

B, L, D = 2, 2048, 2048
Q_HEADS, KV_HEADS, DK, DV = 16, 4, 128, 128
GROUPS = Q_HEADS // KV_HEADS
P = 128
CH = 512                 # lq/lk block width
NJ = L // CH             # 4 query chunks
NDC = D // P             # 16 contraction tiles
NB = L // CH             # 4 lk blocks for vproj
SCALE_EXP = 1.0 / math.sqrt(DK)
EBIAS = -8.0 * math.log(2.0)   # exp output scaled by 2^-8; cancels in softmax
VT_E3 = True             # V^T as e3m4 single plane (False -> fp16)
VT_DT, VT_NP = (E3, NE3) if VT_E3 else (FP16, np.float16)


def _build():
    nc = bacc.Bacc(trn_type="TRN2")
    kt_d = nc.dram_tensor("kt", (D, L), FP16, kind="ExternalInput")
    qt_d = nc.dram_tensor("qt", (NJ, D, CH), FP16, kind="ExternalInput")
    vt_d = nc.dram_tensor("vt", (NB, D, CH), VT_DT, kind="ExternalInput")
    wq_d = nc.dram_tensor("wq", (P, NDC, DK), FP16, kind="ExternalInput")
    wk_d = nc.dram_tensor("wk", (P, NDC, DK), FP16, kind="ExternalInput")
    wv_d = nc.dram_tensor("wv", (P, NDC, DV), FP16, kind="ExternalInput")
    wo_d = nc.dram_tensor("wo", (DV, D), FP16, kind="ExternalInput")
    mask_d = nc.dram_tensor("mask", (P, NJ * CH), FP16, kind="ExternalInput")
    y_d = nc.dram_tensor("y", (L, D), FP16, kind="ExternalOutput")

    with tile.TileContext(nc) as tc:
        with (
            tc.tile_pool(name="const", bufs=1) as const,
            tc.tile_pool(name="wpool", bufs=1) as wpool,
            tc.tile_pool(name="kxs", bufs=5) as kxs,
            tc.tile_pool(name="qxs", bufs=2) as qxs,
            tc.tile_pool(name="vxs", bufs=2) as vxs,
            tc.tile_pool(name="proj", bufs=1) as proj,
            tc.tile_pool(name="etp", bufs=2) as etp,
            tc.tile_pool(name="ev", bufs=3) as ev_pool,
            tc.tile_pool(name="ps", bufs=7, space="PSUM") as ps,
        ):
            ident = const.tile([P, P], FP16)
            make_identity(nc, ident[:])
            ones = const.tile([P, P], FP16)
            nc.vector.memset(ones[:], 1.0)
            ones2 = const.tile([P, 256], FP16)
            nc.vector.memset(ones2[:], 1.0)
            ebias = const.tile([P, 1], FP32)
            nc.vector.memset(ebias[:], EBIAS)
            maskt = const.tile([P, NJ * CH], FP16)

            kT = proj.tile([P, L], FP16, tag="kT")
            qT = proj.tile([P, L], FP16, tag="qT")
            vT = proj.tile([P, L], FP16, tag="vT")
            v_nat = proj.tile([P, L], FP16, tag="v_nat")
            oT = proj.tile([P, L], FP16, tag="oT")
            rinv_all = proj.tile([P, NJ * CH], FP32, tag="rinv_all")

            wq = wpool.tile([P, NDC, DK], FP16, tag="wq")
            wk = wpool.tile([P, NDC, DK], FP16, tag="wk")
            wv = wpool.tile([P, NDC, DV], FP16, tag="wv")
            wo_sb = wpool.tile([DV, D], FP16, tag="wo")

            warm = ps.tile([P, 256], FP32, tag="warm", bufs=1, name="warm")

            def fill():
                # keep the PE HAM window busy / p-state high during DMA waits
                nc.tensor.matmul(warm[:], ones[:], ones2[:], start=True, stop=True)

            # ---------- phase A: K projection (row-major dc stream) ---------
            nc.scalar.dma_start(wk[:], wk_d[:])
            nc.scalar.dma_start(maskt[:], mask_d[:])
            kaccs = [ps.tile([P, CH], FP32, tag="ps", name=f"kacc{b}")
                     for b in range(4)]
            for dc in range(NDC):
                xt = kxs.tile([P, L], FP16, tag="kx", name="kx")
                nc.sync.dma_start(xt[:], kt_d[dc * P:(dc + 1) * P, :])
                for blk in range(4):
                    nc.tensor.matmul(kaccs[blk][:], wk[:, dc, :],
                                     xt[:, blk * CH:(blk + 1) * CH],
                                     start=(dc == 0), stop=(dc == NDC - 1))
                fill()
            for blk in range(4):
                nc.vector.tensor_copy(kT[:, blk * CH:(blk + 1) * CH], kaccs[blk][:])

            # ---------- phase B: V projection (column-block stream) ---------
            nc.scalar.dma_start(wv[:], wv_d[:])
            for b in range(NB):
                vx = vxs.tile([P, NDC, CH], VT_DT, tag="vx", name="vx")
                for dc in range(NDC):
                    nc.sync.dma_start(vx[:, dc, :], vt_d[b, dc * P:(dc + 1) * P, :])
                vacc = ps.tile([P, CH], FP32, tag="ps", name=f"vacc{b}")
                for dc in range(NDC):
                    nc.tensor.matmul(vacc[:], wv[:, dc, :], vx[:, dc, :],
                                     start=(dc == 0), stop=(dc == NDC - 1))
                nc.vector.tensor_copy(vT[:, b * CH:(b + 1) * CH], vacc[:])
                for c in range(4 * b, 4 * b + 4):
                    tp = ps.tile([P, P], FP16, tag="ps", name="tp")
                    nc.tensor.transpose(tp[:], vT[:, c * P:(c + 1) * P], ident[:])
                    nc.scalar.copy(v_nat[:, c * P:(c + 1) * P], tp[:])
                fill()

            # ---------- phase C: per-j chains -------------------------------
            nc.scalar.dma_start(wq[:], wq_d[:])
            nc.scalar.dma_start(wo_sb[:], wo_d[:])

            ets = {}

            def qproj(j):
                qx = qxs.tile([P, NDC, CH], FP16, tag="qx", name="qx")
                for dc in range(NDC):
                    nc.sync.dma_start(qx[:, dc, :], qt_d[j, dc * P:(dc + 1) * P, :])
                qacc = ps.tile([P, CH], FP32, tag="ps", name=f"qacc{j}")
                for dc in range(NDC):
                    nc.tensor.matmul(qacc[:], wq[:, dc, :], qx[:, dc, :],
                                     start=(dc == 0), stop=(dc == NDC - 1))
                nc.vector.tensor_copy(qT[:, j * CH:(j + 1) * CH], qacc[:])

            def scores(j):
                et_all = etp.tile([P, NDC, CH], FP16, tag="et", name="et")
                ets[j] = et_all
                rrep = ps.tile([P, CH], FP32, tag="ps", name=f"rrep{j}")
                for c in range(4 * j + 4):
                    st = ps.tile([P, CH], FP32, tag="ps", name="st")
                    nc.tensor.matmul(st[:], kT[:, c * P:(c + 1) * P],
                                     qT[:, j * CH:(j + 1) * CH],
                                     start=True, stop=True)
                    et = et_all[:, c, :]
                    nc.scalar.activation(et, st[:],
                                         mybir.ActivationFunctionType.Exp,
                                         bias=ebias[:], scale=SCALE_EXP)
                    d = c - 4 * j
                    if d >= 0:   # diagonal tile: zero out k > q
                        nc.vector.tensor_mul(et, et, maskt[:, d * CH:(d + 1) * CH])
                    nc.tensor.matmul(rrep[:], ones[:], et,
                                     start=(c == 0), stop=(c == 4 * j + 3))
                rinv = rinv_all[:, j * CH:(j + 1) * CH]
                nc.vector.reciprocal_approx_fast(rinv, rrep[:])

            def pv(j):
                et_all = ets[j]
                ot = ps.tile([P, CH], FP32, tag="ps", name="ot")
                for c in range(4 * j + 4):
                    nc.tensor.matmul(ot[:], v_nat[:, c * P:(c + 1) * P],
                                     et_all[:, c, :],
                                     start=(c == 0), stop=(c == 4 * j + 3))
                nc.vector.tensor_mul(oT[:, j * CH:(j + 1) * CH], ot[:],
                                     rinv_all[:, j * CH:(j + 1) * CH])

            def y_chunk(j):
                for t in range(CH // P):
                    lq0 = j * CH + t * P
                    yev = ev_pool.tile([P, D], FP16, tag="yev", name="yev")
                    for dch in range(D // CH):
                        yps = ps.tile([P, CH], FP32, tag="ps", name="yps")
                        nc.tensor.matmul(yps[:], oT[:, lq0:lq0 + P],
                                         wo_sb[:, dch * CH:(dch + 1) * CH],
                                         start=True, stop=True)
                        dst = yev[:, dch * CH:(dch + 1) * CH]
                        if dch % 2 == 0:
                            nc.vector.tensor_copy(dst, yps[:])
                        else:
                            nc.scalar.copy(dst, yps[:])
                    # gpsimd queue: keeps the sync queue free for qt prefetch
                    nc.gpsimd.dma_start(y_d[lq0:lq0 + P, :], yev[:])

            qproj(0)
            scores(0)
            for j in range(1, NJ):
                qproj(j)
                pv(j - 1)
                scores(j)
                y_chunk(j - 1)
            pv(NJ - 1)
            y_chunk(NJ - 1)
    nc.compile()
    return nc


_NC = None


def _get_nc():
    global _NC
    if _NC is None:
        _NC = _build()
    return _NC


def _pack_w(w):
    """(D, dk) fp32 -> [P, NDC, dk] fp16: out[p, dc, m] = w[dc*128+p, m]"""
    return np.ascontiguousarray(
        w.reshape(NDC, P, -1).transpose(1, 0, 2)).astype(np.float16)


def _col_blocks(xt, dt):
    """[D, L] -> contiguous (NJ, D, CH) in dtype dt."""
    return np.ascontiguousarray(
        xt.reshape(D, NJ, CH).transpose(1, 0, 2)).astype(dt)


def _make_in_maps(Q, K, V, Wq, Wk, Wv, Wo):
    f16 = np.float16
    Wq_eff = np.asarray(Wq, np.float32).reshape(D, GROUPS, KV_HEADS, DK).sum(axis=1)
    mask = np.zeros((P, NJ * CH), f16)
    for d in range(4):
        p = np.arange(P)[:, None]
        x = np.arange(CH)[None, :]
        mask[:, d * CH:(d + 1) * CH] = (128 * d + p <= x).astype(f16)
    acts = {}
    for b in range(B):
        qt = np.ascontiguousarray(np.asarray(Q[b], np.float32).T)
        kt = np.ascontiguousarray(np.asarray(K[b], np.float32).T)
        vt = np.ascontiguousarray(np.asarray(V[b], np.float32).T)
        acts[b] = {
            "kt": kt.astype(f16),
            "qt": _col_blocks(qt, f16),
            "vt": _col_blocks(vt, VT_NP),
        }
    Wk32, Wv32 = np.asarray(Wk, np.float32), np.asarray(Wv, np.float32)
    Wo32 = np.asarray(Wo, np.float32)
    in_maps = []
    for c in range(8):
        b, h = divmod(c, KV_HEADS)
        in_maps.append({
            **acts[b],
            "wq": _pack_w(Wq_eff[:, h, :]),
            "wk": _pack_w(Wk32[:, h * DK:(h + 1) * DK]),
            "wv": _pack_w(Wv32[:, h * DV:(h + 1) * DV]),
            "wo": Wo32[h * DV:(h + 1) * DV, :].astype(f16),
            "mask": mask,
        })
    return in_maps


def _gather(results):
    Y = np.zeros((B, L, D), np.float32)
    for c in range(8):
        Y[c // KV_HEADS] += results[c]["y"].astype(np.float32)
    return Y


def kernel(Q, K, V, Wq, Wk, Wv, Wo):
    nc = _get_nc()
    in_maps = _make_in_maps(Q, K, V, Wq, Wk, Wv, Wo)
    res = bass_utils.run_bass_kernel_spmd(nc, in_maps, core_ids=list(range(8)))
    return _gather(res.results)


def _install_ntff_hook():
    """The agent image's antenv lacks axon_hooks; synthesize it so
    trace=True can reach the NTFF profiler in libaxon_pjrt.so."""
    import types
    import antenv
    if hasattr(antenv, "axon_hooks"):
        return
    mod = types.ModuleType("antenv.axon_hooks")
    _h = [None]
    mod.set_axon_ntff_profile_hook = lambda h: _h.__setitem__(0, h)
    mod.get_axon_ntff_profile_hook = lambda: _h[0]
    sys.modules["antenv.axon_hooks"] = mod
    antenv.axon_hooks = mod
    from trn_agent_boot.trn_boot import _ntff_profile_via_ctypes
    mod.set_axon_ntff_profile_hook(_ntff_profile_via_ctypes("/opt/axon/libaxon_pjrt.so"))


def kernel_traced(Q, K, V, Wq, Wk, Wv, Wo):
    """Like kernel() but profiles; returns (output, BassKernelResults)."""
    _install_ntff_hook()
    nc = _get_nc()
    in_maps = _make_in_maps(Q, K, V, Wq, Wk, Wv, Wo)
    res = bass_utils.run_bass_kernel_spmd(nc, in_maps, core_ids=list(range(8)),
                                          trace=True)
    return _gather(res.results), res


# revision 10
# speedup vs baseline: 1.4595x; 1.2181x over previous
"""GQA kernel for Trainium2, sharded over 8 NeuronCores.

Sharding: data-parallel over batch (2) x tensor-parallel over kv_heads (4).
Core c = b*4 + h computes the full attention output partial
    Y_bh = softmax(causal((Q_b @ Wq_eff_h) @ (K_b @ Wk_h)^T / sqrt(dk))) @ (V_b @ Wv_h) @ Wo_h
and the host sums the 4 head partials per batch (the "all-reduce after Wo").
The GQA group-sum-before-softmax quirk folds into the weights:
    Wq_eff_h = sum_g Wq[:, (g*KV+h)*dk : ...].

Bandwidth plan (validated in simulation + on hw):
  - V^T streams as a single e3m4 plane (half the bytes of fp16); wv fp16
    stationary (mixed-dtype matmul, verified on hw). Total max rel err
    1.58e-2, under the 2e-2 gate. VT_E3=False falls back to fp16 V.
  - Everything else fp16 with fp32 PSUM (DoubleRow fp8 measured to give
    no speedup on this hw, so 2-plane fp8 tricks are pointless).

Schedule: K^T arrives row-major (kproj chases the wire); V^T and Q^T
arrive column-block-major (512 positions per block) so vT blocks and
qT chunks complete early; per q-chunk j the chain
qproj(j) -> scores(j,*) -> PV(j-1) -> Y(j-1) -> y DMA runs as soon as
block j lands, overlapping the y output DMA with the input stream tail.
DMA wire is the binding resource (~32MB at ~350GB/s).
"""
import sys
sys.path.insert(0, '/opt/trn_rl_repo')
import math
import numpy as np
import ml_dtypes

import concourse.bass as bass
import concourse.mybir as mybir
import concourse.tile as tile
from concourse import bacc
from concourse import bass_utils
from concourse.masks import make_identity

FP32 = mybir.dt.float32
FP16 = mybir.dt.float16
E3 = mybir.dt.float8e3
NE3 = ml_dtypes.float8_e3m4

B, L, D = 2, 2048, 2048
Q_HEADS, KV_HEADS, DK, DV = 16, 4, 128, 128
GROUPS = Q_HEADS // KV_HEADS
P = 128
CH = 512                 # lq/lk block width
NJ = L // CH             # 4 query chunks
NDC = D // P             # 16 contraction tiles
NB = L // CH             # 4 lk blocks for vproj
SCALE_EXP = 1.0 / math.sqrt(DK)
EBIAS = -8.0 * math.log(2.0)   # exp output scaled by 2^-8; cancels in softmax
VT_E3 = True             # V^T as e3m4 single plane (False -> fp16)
VT_DT, VT_NP = (E3, NE3) if VT_E3 else (FP16, np.float16)


def _build():
    nc = bacc.Bacc(trn_type="TRN2")
    kt_d = nc.dram_tensor("kt", (D, L), FP16, kind="ExternalInput")
    qt_d = nc.dram_tensor("qt", (NJ, D, CH), FP16, kind="ExternalInput")
    vt_d = nc.dram_tensor("vt", (NB, D, CH), VT_DT, kind="ExternalInput")
    wq_d = nc.dram_tensor("wq", (P, NDC, DK), FP16, kind="ExternalInput")
    wk_d = nc.dram_tensor("wk", (P, NDC, DK), FP16, kind="ExternalInput")
    wv_d = nc.dram_tensor("wv", (P, NDC, DV), FP16, kind="ExternalInput")
    wo_d = nc.dram_tensor("wo", (DV, D), FP16, kind="ExternalInput")
    mask_d = nc.dram_tensor("mask", (P, NJ * CH), FP16, kind="ExternalInput")
    y_d = nc.dram_tensor("y", (L, D), FP16, kind="ExternalOutput")

    with tile.TileContext(nc) as tc:
        with (
            tc.tile_pool(name="const", bufs=1) as const,
            tc.tile_pool(name="wpool", bufs=1) as wpool,
            tc.tile_pool(name="kxs", bufs=5) as kxs,
            tc.tile_pool(name="qxs", bufs=2) as qxs,
            tc.tile_pool(name="vxs", bufs=2) as vxs,
            tc.tile_pool(name="proj", bufs=1) as proj,
            tc.tile_pool(name="etp", bufs=2) as etp,
            tc.tile_pool(name="ev", bufs=3) as ev_pool,
            tc.tile_pool(name="ps", bufs=7, space="PSUM") as ps,
        ):
            ident = const.tile([P, P], FP16)
            make_identity(nc, ident[:])
            ones = const.tile([P, P], FP16)
            nc.vector.memset(ones[:], 1.0)
            ones2 = const.tile([P, 256], FP16)
            nc.vector.memset(ones2[:], 1.0)
            ebias = const.tile([P, 1], FP32)
            nc.vector.memset(ebias[:], EBIAS)
            maskt = const.tile([P, NJ * CH], FP16)

            kT = proj.tile([P, L], FP16, tag="kT")
            qT = proj.tile([P, L], FP16, tag="qT")
            vT = proj.tile([P, L], FP16, tag="vT")
            v_nat = proj.tile([P, L], FP16, tag="v_nat")
            oT = proj.tile([P, L], FP16, tag="oT")
            rinv_all = proj.tile([P, NJ * CH], FP32, tag="rinv_all")

            wq = wpool.tile([P, NDC, DK], FP16, tag="wq")
            wk = wpool.tile([P, NDC, DK], FP16, tag="wk")
            wv = wpool.tile([P, NDC, DV], FP16, tag="wv")
            wo_sb = wpool.tile([DV, D], FP16, tag="wo")

            warm = ps.tile([P, 256], FP32, tag="warm", bufs=1, name="warm")

            def fill():
                # keep the PE HAM window busy / p-state high during DMA waits
                nc.tensor.matmul(warm[:], ones[:], ones2[:], start=True, stop=True)

            # ---------- phase A: K projection (row-major dc stream) ---------
            nc.scalar.dma_start(wk[:], wk_d[:])
            nc.scalar.dma_start(maskt[:], mask_d[:])
            kaccs = [ps.tile([P, CH], FP32, tag="ps", name=f"kacc{b}")
                     for b in range(4)]
            for dc in range(NDC):
                xt = kxs.tile([P, L], FP16, tag="kx", name="kx")
                nc.sync.dma_start(xt[:], kt_d[dc * P:(dc + 1) * P, :])
                for blk in range(4):
                    nc.tensor.matmul(kaccs[blk][:], wk[:, dc, :],
                                     xt[:, blk * CH:(blk + 1) * CH],
                                     start=(dc == 0), stop=(dc == NDC - 1))
                fill()
            for blk in range(4):
                nc.vector.tensor_copy(kT[:, blk * CH:(blk + 1) * CH], kaccs[blk][:])

            # ---------- phase B: V projection (column-block stream) ---------
            nc.scalar.dma_start(wv[:], wv_d[:])
            for b in range(NB):
                vx = vxs.tile([P, NDC, CH], VT_DT, tag="vx", name="vx")
                nc.sync.dma_start(
                    vx[:], vt_d[b].rearrange("(dc p) c -> p dc c", p=P))
                vacc = ps.tile([P, CH], FP32, tag="ps", name=f"vacc{b}")
                for dc in range(NDC):
                    nc.tensor.matmul(vacc[:], wv[:, dc, :], vx[:, dc, :],
                                     start=(dc == 0), stop=(dc == NDC - 1))
                nc.vector.tensor_copy(vT[:, b * CH:(b + 1) * CH], vacc[:])
                for c in range(4 * b, 4 * b + 4):
                    tp = ps.tile([P, P], FP16, tag="ps", name="tp")
                    nc.tensor.transpose(tp[:], vT[:, c * P:(c + 1) * P], ident[:])
                    nc.scalar.copy(v_nat[:, c * P:(c + 1) * P], tp[:])
                fill()

            # ---------- phase C: per-j chains -------------------------------
            nc.scalar.dma_start(wq[:], wq_d[:])
            nc.scalar.dma_start(wo_sb[:], wo_d[:])

            ets = {}

            def qproj(j):
                qx = qxs.tile([P, NDC, CH], FP16, tag="qx", name="qx")
                nc.sync.dma_start(
                    qx[:], qt_d[j].rearrange("(dc p) c -> p dc c", p=P))
                qacc = ps.tile([P, CH], FP32, tag="ps", name=f"qacc{j}")
                for dc in range(NDC):
                    nc.tensor.matmul(qacc[:], wq[:, dc, :], qx[:, dc, :],
                                     start=(dc == 0), stop=(dc == NDC - 1))
                nc.vector.tensor_copy(qT[:, j * CH:(j + 1) * CH], qacc[:])

            def scores(j):
                et_all = etp.tile([P, NDC, CH], FP16, tag="et", name="et")
                ets[j] = et_all
                rrep = ps.tile([P, CH], FP32, tag="ps", name=f"rrep{j}")
                for c in range(4 * j + 4):
                    st = ps.tile([P, CH], FP32, tag="ps", name="st")
                    nc.tensor.matmul(st[:], kT[:, c * P:(c + 1) * P],
                                     qT[:, j * CH:(j + 1) * CH],
                                     start=True, stop=True)
                    et = et_all[:, c, :]
                    nc.scalar.activation(et, st[:],
                                         mybir.ActivationFunctionType.Exp,
                                         bias=ebias[:], scale=SCALE_EXP)
                    d = c - 4 * j
                    if d >= 0:   # diagonal tile: zero out k > q
                        nc.vector.tensor_mul(et, et, maskt[:, d * CH:(d + 1) * CH])
                    nc.tensor.matmul(rrep[:], ones[:], et,
                                     start=(c == 0), stop=(c == 4 * j + 3))
                rinv = rinv_all[:, j * CH:(j + 1) * CH]
                nc.vector.reciprocal_approx_fast(rinv, rrep[:])

            def pv(j):
                et_all = ets[j]
                ot = ps.tile([P, CH], FP32, tag="ps", name="ot")
                for c in range(4 * j + 4):
                    nc.tensor.matmul(ot[:], v_nat[:, c * P:(c + 1) * P],
                                     et_all[:, c, :],
                                     start=(c == 0), stop=(c == 4 * j + 3))
                nc.vector.tensor_mul(oT[:, j * CH:(j + 1) * CH], ot[:],
                                     rinv_all[:, j * CH:(j + 1) * CH])

            def y_chunk(j):
                for t in range(CH // P):
                    lq0 = j * CH + t * P
                    yev = ev_pool.tile([P, D], FP16, tag="yev", name="yev")
                    for dch in range(D // CH):
                        yps = ps.tile([P, CH], FP32, tag="ps", name="yps")
                        nc.tensor.matmul(yps[:], oT[:, lq0:lq0 + P],
                                         wo_sb[:, dch * CH:(dch + 1) * CH],
                                         start=True, stop=True)
                        dst = yev[:, dch * CH:(dch + 1) * CH]
                        if dch % 2 == 0:
                            nc.vector.tensor_copy(dst, yps[:])
                        else:
                            nc.scalar.copy(dst, yps[:])
                    # gpsimd queue: keeps the sync queue free for qt prefetch
                    nc.gpsimd.dma_start(y_d[lq0:lq0 + P, :], yev[:])

            qproj(0)
            scores(0)
            for j in range(1, NJ):
                qproj(j)
                pv(j - 1)
                scores(j)
                y_chunk(j - 1)
            pv(NJ - 1)
            y_chunk(NJ - 1)
    nc.compile()
    return nc


_NC = None


def _get_nc():
    global _NC
    if _NC is None:
        _NC = _build()
    return _NC


def _pack_w(w):
    """(D, dk) fp32 -> [P, NDC, dk] fp16: out[p, dc, m] = w[dc*128+p, m]"""
    return np.ascontiguousarray(
        w.reshape(NDC, P, -1).transpose(1, 0, 2)).astype(np.float16)


def _col_blocks(xt, dt):
    """[D, L] -> contiguous (NJ, D, CH) in dtype dt."""
    return np.ascontiguousarray(
        xt.reshape(D, NJ, CH).transpose(1, 0, 2)).astype(dt)


def _make_in_maps(Q, K, V, Wq, Wk, Wv, Wo):
    f16 = np.float16
    Wq_eff = np.asarray(Wq, np.float32).reshape(D, GROUPS, KV_HEADS, DK).sum(axis=1)
    mask = np.zeros((P, NJ * CH), f16)
    for d in range(4):
        p = np.arange(P)[:, None]
        x = np.arange(CH)[None, :]
        mask[:, d * CH:(d + 1) * CH] = (128 * d + p <= x).astype(f16)
    acts = {}
    for b in range(B):
        qt = np.ascontiguousarray(np.asarray(Q[b], np.float32).T)
        kt = np.ascontiguousarray(np.asarray(K[b], np.float32).T)
        vt = np.ascontiguousarray(np.asarray(V[b], np.float32).T)
        acts[b] = {
            "kt": kt.astype(f16),
            "qt": _col_blocks(qt, f16),
            "vt": _col_blocks(vt, VT_NP),
        }
    Wk32, Wv32 = np.asarray(Wk, np.float32), np.asarray(Wv, np.float32)
    Wo32 = np.asarray(Wo, np.float32)
    in_maps = []
    for c in range(8):
        b, h = divmod(c, KV_HEADS)
        in_maps.append({
            **acts[b],
            "wq": _pack_w(Wq_eff[:, h, :]),
            "wk": _pack_w(Wk32[:, h * DK:(h + 1) * DK]),
            "wv": _pack_w(Wv32[:, h * DV:(h + 1) * DV]),
            "wo": Wo32[h * DV:(h + 1) * DV, :].astype(f16),
            "mask": mask,
        })
    return in_maps


def _gather(results):
    Y = np.zeros((B, L, D), np.float32)
    for c in range(8):
        Y[c // KV_HEADS] += results[c]["y"].astype(np.float32)
    return Y


def kernel(Q, K, V, Wq, Wk, Wv, Wo):
    nc = _get_nc()
    in_maps = _make_in_maps(Q, K, V, Wq, Wk, Wv, Wo)
    res = bass_utils.run_bass_kernel_spmd(nc, in_maps, core_ids=list(range(8)))
    return _gather(res.results)


def _install_ntff_hook():
    """The agent image's antenv lacks axon_hooks; synthesize it so
    trace=True can reach the NTFF profiler in libaxon_pjrt.so."""
    import types
    import antenv
    if hasattr(antenv, "axon_hooks"):
        return
    mod = types.ModuleType("antenv.axon_hooks")
    _h = [None]
    mod.set_axon_ntff_profile_hook = lambda h: _h.__setitem__(0, h)
    mod.get_axon_ntff_profile_hook = lambda: _h[0]
    sys.modules["antenv.axon_hooks"] = mod
    antenv.axon_hooks = mod
    from trn_agent_boot.trn_boot import _ntff_profile_via_ctypes
    mod.set_axon_ntff_profile_hook(_ntff_profile_via_ctypes("/opt/axon/libaxon_pjrt.so"))


def kernel_traced(Q, K, V, Wq, Wk, Wv, Wo):
    """Like kernel() but profiles; returns (output, BassKernelResults)."""
    _install_ntff_hook()
    nc = _get_nc()
    in_maps = _make_in_maps(Q, K, V, Wq, Wk, Wv, Wo)
    res = bass_utils.run_bass_kernel_spmd(nc, in_maps, core_ids=list(range(8)),
                                          trace=True)
    return _gather(res.results), res
